# revision 1
# baseline (speedup 1.0000x reference)
"""Multi-head attention (N=2, S=2048, D=1024, H=16) on 8 TRN2 NeuronCores.

Sharding: core c handles batch b = c//4 and head group g = c%4 (4 heads).
Column-parallel qkv projection, per-head attention, row-parallel o_proj;
the 4 partial o_proj outputs per batch are summed on the host (unshard).

Per-core device kernel (bf16 matmul operands, fp32 PSUM accumulate):
  phase 1: qkT = wqkT.T @ xT   (q/k transposed layouts, head pairs stacked
           on partitions), v = xT.T @ wvT (natural layout, with a ones
           column appended per head for fused softmax-denominator)
  phase 2: per head pair / sq-block: scoresT = kT.T @ qT -> exp (ACT,
           fused 1/sqrt(hd) scale) -> valuesT(+denom) = v_ext.T @ attnT
  phase 3: divide by denom (reciprocal + PE ones-broadcast + DVE mul)
  phase 4: o_part = valuesT.T @ owT -> DMA out
"""

import numpy as np

import concourse.bass as bass  # noqa: F401
import concourse.mybir as mybir
import concourse.tile as tile
from concourse import bacc
from concourse.bass_utils import run_bass_kernel_spmd

f32 = mybir.dt.float32
f32r = mybir.dt.float32r
bf16 = mybir.dt.bfloat16
AF = mybir.ActivationFunctionType

import os as _os
MM_DT = f32r if _os.environ.get("MM_DT", "bf16") == "f32r" else bf16

P = 128
N, S, D, H = 2, 2048, 1024, 16
HD = D // H                    # 64
NH = 4                         # heads per core
SCALE = float(1.0 / np.sqrt(np.float32(HD)))
E_QK = 2 * NH * HD             # 512 qk rows per core
E_V = NH * HD                  # 256
DL = NH * HD                   # 256 local d for o_proj
SQB = 512                      # sq block
NSQB = S // SQB                # 4
SKT = S // P                   # 16 sk tiles

# tuning knobs
SKG = int(_os.environ.get("SKG", "1"))        # sk tiles per exp group
SC_BUFS = int(_os.environ.get("SC_BUFS", "2"))
VP_BUFS = int(_os.environ.get("VP_BUFS", "3"))
MP_BUFS = int(_os.environ.get("MP_BUFS", "1"))
ATTN_BUFS = int(_os.environ.get("ATTN_BUFS", "4"))
LAG = int(_os.environ.get("LAG", "2"))   # values matmul lag behind exp


def _emit_body(nc, tc, t, rep):
    from contextlib import ExitStack

    with ExitStack() as ctx:
        const = ctx.enter_context(tc.tile_pool(name=f"const{rep}", bufs=1))
        persist = ctx.enter_context(tc.tile_pool(name=f"persist{rep}", bufs=1))

        wqk_sb = const.tile([P, 8, E_QK], MM_DT, name="wqk_sb")
        wqk_r = t["wqkT"].rearrange("(a p) e -> p a e", p=P)
        for a in range(8):
            nc.scalar.dma_start(wqk_sb[:, a, :], wqk_r[:, a, :])
        wv_sb = const.tile([P, 8, E_V], MM_DT, name="wv_sb")
        nc.gpsimd.dma_start(wv_sb[:], t["wvT"].rearrange("(a p) e -> p a e", p=P))
        ow_sb = const.tile([P, 2, D], MM_DT, name="ow_sb")
        ones_sb = const.tile([65, HD], MM_DT, name="ones_sb")

        qT = persist.tile([P, 2, S], MM_DT, name="qT")
        kT = persist.tile([P, 2, S], MM_DT, name="kT")
        v_sb = persist.tile([P, SKT, NH * 65], MM_DT, name="v_sb")
        vals = persist.tile([P, 2, S], MM_DT, name="vals")
        nc.sync.dma_start(
            v_sb.rearrange("p a (h e) -> p a h e", e=65)[:, :, :, 64:65],
            t["onesd"].rearrange("p (a h) -> p a h", h=NH)[:, :, :, None],
        )

        # ---------------- phase 1: projections ----------------
        with (
            tc.tile_pool(name=f"xp{rep}", bufs=2) as xp,
            tc.tile_pool(name=f"ps1{rep}", bufs=3, space="PSUM") as ps1,
        ):
            xT_r = t["xT"].rearrange("(a p) s -> p a s", p=P)
            for sb in range(4):
                ss = slice(sb * 512, (sb + 1) * 512)
                xt = xp.tile([P, 8, 512], MM_DT, name="xt")
                for a in range(8):
                    eng = nc.gpsimd if a % 2 == 0 else nc.scalar
                    eng.dma_start(xt[:, a, :], xT_r[:, a, ss])
                for et in range(4):
                    pq = ps1.tile([P, 512], f32, name="pq", tag="pq")
                    for a in range(8):
                        nc.tensor.matmul(
                            pq[:],
                            wqk_sb[:, a, et * 128:(et + 1) * 128],
                            xt[:, a, :],
                            start=(a == 0),
                            stop=(a == 7),
                        )
                    dst = (qT if et % 2 == 0 else kT)[:, et // 2, ss]
                    nc.any.tensor_copy(dst, pq[:])
                for st in range(4):
                    pv = ps1.tile([P, E_V], f32, name="pv", tag="pv")
                    for a in range(8):
                        nc.tensor.matmul(
                            pv[:],
                            xt[:, a, st * 128:(st + 1) * 128],
                            wv_sb[:, a, :],
                            start=(a == 0),
                            stop=(a == 7),
                        )
                    so = sb * 4 + st
                    v_r = v_sb[:, so].rearrange("p (h e) -> p h e", e=65)
                    pv_r = pv.rearrange("p (h e) -> p h e", e=64)
                    nc.any.tensor_copy(v_r[:, :, 0:64], pv_r)
                    nc.vector.tensor_scalar(
                        out=v_r[:, :, 64:65],
                        in0=pv_r[:, :, 0:1],
                        scalar1=0.0,
                        scalar2=1.0,
                        op0=mybir.AluOpType.mult,
                        op1=mybir.AluOpType.add,
                    )

        # ---------------- phase 2-4: attention + o_proj ----------------
        nc.sync.dma_start(ow_sb[:], t["owT"].rearrange("(a p) e -> p a e", p=P))
        nc.sync.dma_start(ones_sb[64:65, :], t["onesd"][64:65, 0:HD])
        with (
            tc.tile_pool(name=f"scp{rep}", bufs=SC_BUFS, space="PSUM") as scp,
            tc.tile_pool(name=f"vp{rep}", bufs=VP_BUFS, space="PSUM") as vp,
            tc.tile_pool(name=f"mp{rep}", bufs=MP_BUFS, space="PSUM") as mp,
            tc.tile_pool(name=f"attn{rep}", bufs=ATTN_BUFS) as attnp,
            tc.tile_pool(name=f"sm{rep}", bufs=2) as sm,
            tc.tile_pool(name=f"outp{rep}", bufs=3) as outp,
        ):
            for qb in range(NSQB):
                sqs = slice(qb * SQB, (qb + 1) * SQB)
                for pr in range(2):
                    vps = [
                        vp.tile([65, SQB], f32, name=f"vps{h}", tag="vps")
                        for h in range(2)
                    ]
                    at_tiles = {}
                    ngroups = SKT // SKG
                    for g in range(ngroups + LAG):
                        if g < ngroups:
                            sc = scp.tile([P, SKG, 2, SQB], f32, name="sc",
                                          tag="sc")
                            at = attnp.tile([P, SKG, 2, SQB], MM_DT, name="at",
                                            tag="at")
                            for j in range(SKG):
                                sk = g * SKG + j
                                for h in range(2):
                                    nc.tensor.matmul(
                                        sc[:, j, h, :],
                                        kT[h * 64:(h + 1) * 64, pr,
                                           sk * 128:(sk + 1) * 128],
                                        qT[h * 64:(h + 1) * 64, pr, sqs],
                                        start=True,
                                        stop=True,
                                    )
                            nc.scalar.activation(at[:], sc[:], AF.Exp,
                                                 scale=SCALE)
                            at_tiles[g] = at
                        if g >= LAG:
                            gg = g - LAG
                            atv = at_tiles.pop(gg)
                            for j in range(SKG):
                                sk = gg * SKG + j
                                for h in range(2):
                                    lh = pr * 2 + h
                                    nc.tensor.matmul(
                                        vps[h][:],
                                        v_sb[:, sk, lh * 65:(lh + 1) * 65],
                                        atv[:, j, h, :],
                                        start=(sk == 0),
                                        stop=(sk == SKT - 1),
                                    )
                    for h in range(2):
                        rec_f = sm.tile([65, SQB], f32, name="rec_f", tag="rec_f")
                        nc.vector.reciprocal_approx_fast(
                            rec_f[0:65, :], vps[h][0:65, :]
                        )
                        recr = sm.tile([65, SQB], MM_DT, name="recr", tag="recr")
                        nc.vector.tensor_copy(recr[64:65, :], rec_f[64:65, :])
                        bc = mp.tile([P, SQB], f32, name="bc", tag="m")[0:64, :]
                        nc.tensor.matmul(
                            bc[:],
                            ones_sb[64:65, :],
                            recr[64:65, :],
                            start=True,
                            stop=True,
                        )
                        bcs = sm.tile([64, SQB], f32, name="bcs", tag="bcs")
                        nc.vector.tensor_copy(bcs[:], bc[:])
                        if h == 0:
                            nc.vector.tensor_mul(
                                out=vals[0:64, pr, sqs],
                                in0=vps[h][0:64, :],
                                in1=bcs[:],
                            )
                        else:
                            tmp = sm.tile([64, SQB], MM_DT, name="tmpv", tag="tmpv")
                            nc.vector.tensor_mul(
                                out=tmp[:], in0=vps[h][0:64, :], in1=bcs[:]
                            )
                            nc.sync.dma_start(vals[64:128, pr, sqs], tmp[:])
                # o_proj for the 4 s-tiles of this q block
                for st in range(4):
                    s0 = qb * 4 + st
                    for eb in range(2):
                        ops = mp.tile([P, 512], f32, name="ops", tag="m")
                        for a in range(2):
                            nc.tensor.matmul(
                                ops[:],
                                vals[:, a, s0 * 128:(s0 + 1) * 128],
                                ow_sb[:, a, eb * 512:(eb + 1) * 512],
                                start=(a == 0),
                                stop=(a == 1),
                            )
                        ot = outp.tile([P, 512], f32, name="ot")
                        nc.vector.tensor_copy(ot[:], ops[:])
                        nc.sync.dma_start(
                            t["o"][s0 * 128:(s0 + 1) * 128,
                                   eb * 512:(eb + 1) * 512],
                            ot[:],
                        )


def build_nc(repeats: int = 1):
    nc = bacc.Bacc(None, target_bir_lowering=False)
    t = {
        "xT": nc.dram_tensor("xT", [D, S], MM_DT, kind="ExternalInput")[:, :],
        "wqkT": nc.dram_tensor("wqkT", [D, E_QK], MM_DT, kind="ExternalInput")[:, :],
        "wvT": nc.dram_tensor("wvT", [D, E_V], MM_DT, kind="ExternalInput")[:, :],
        "owT": nc.dram_tensor("owT", [DL, D], MM_DT, kind="ExternalInput")[:, :],
        "onesd": nc.dram_tensor("onesd", [P, SKT * NH], MM_DT,
                                kind="ExternalInput")[:, :],
        "o": nc.dram_tensor("o", [S, D], f32, kind="ExternalOutput")[:, :],
    }
    with tile.TileContext(nc) as tc:
        for rep in range(repeats):
            _emit_body(nc, tc, t, rep)
    nc.compile()
    return nc


def tf32_round(a):
    if MM_DT == bf16:
        import ml_dtypes
        return np.ascontiguousarray(a, dtype=np.float32).astype(ml_dtypes.bfloat16)
    u = np.ascontiguousarray(a, dtype=np.float32).view(np.uint32)
    r = (u + np.uint32(0xFFF) + ((u >> np.uint32(13)) & np.uint32(1))) & ~np.uint32(
        0x1FFF
    )
    return r.view(np.float32)


def make_in_maps(x, qkv_w, o_w):
    x = np.ascontiguousarray(np.asarray(x, dtype=np.float32))
    qkv_w = np.ascontiguousarray(np.asarray(qkv_w, dtype=np.float32))
    o_w = np.ascontiguousarray(np.asarray(o_w, dtype=np.float32))
    in_maps = []
    for c in range(8):
        b, g = c // 4, c % 4
        heads = [4 * g + i for i in range(NH)]
        xT = np.ascontiguousarray(x[b].T)
        wq = [qkv_w[h * 192:h * 192 + 64] for h in heads]
        wk = [qkv_w[h * 192 + 64:h * 192 + 128] for h in heads]
        wv = [qkv_w[h * 192 + 128:h * 192 + 192] for h in heads]
        wqk = np.concatenate(
            [wq[0], wq[1], wk[0], wk[1], wq[2], wq[3], wk[2], wk[3]], axis=0
        )
        wqkT = np.ascontiguousarray(wqk.T)
        wvT = np.ascontiguousarray(np.concatenate(wv, axis=0).T)
        cols = np.concatenate([np.arange(h * 64, h * 64 + 64) for h in heads])
        owT = np.ascontiguousarray(o_w[:, cols].T)
        in_maps.append({"xT": tf32_round(xT), "wqkT": tf32_round(wqkT),
                        "wvT": tf32_round(wvT), "owT": tf32_round(owT),
                        "onesd": tf32_round(np.ones((P, SKT * NH), np.float32))})
    return in_maps


_NC_CACHE = {}


def _get_nc(repeats=1):
    if repeats not in _NC_CACHE:
        _NC_CACHE[repeats] = build_nc(repeats)
    return _NC_CACHE[repeats]


def run_on_hw(x, qkv_w, o_w, repeats=1, **kwargs):
    nc = _get_nc(repeats)
    in_maps = make_in_maps(x, qkv_w, o_w)
    res = run_bass_kernel_spmd(nc, in_maps, core_ids=list(range(8)), **kwargs)
    out = np.zeros((N, S, D), dtype=np.float32)
    for c in range(8):
        out[c // 4] += res.results[c]["o"]
    return out, res


def kernel(x, qkv_w, o_w):
    out, _ = run_on_hw(x, qkv_w, o_w)
    return out



# revision 18
# speedup vs baseline: 647.5582x; 647.5582x over previous
"""Multi-head attention (N=2, S=2048, D=1024, H=16) on 8 TRN2 NeuronCores.

Sharding: core c handles batch b = c//4 and head group g = c%4 (4 heads).
Column-parallel qkv projection, per-head attention, row-parallel o_proj;
the 4 partial o_proj outputs per batch are summed on the host (unshard).

Per-core device kernel (fp16 matmul operands, fp32 PSUM accumulate).
All engine queues are in-order, so emission order is the schedule: the
kernel is one long software pipeline where normalization (M), o_proj (O)
and the q-projection for the next block are woven one-item-per-sk-tile
into the scores/exp/values stream of a later S-block, giving every
cross-engine chain a full S-block of slack before the PE reaches its
dependent matmul.

  pass A:  kT = wkT.T @ xT and v = xT.T @ wvT for all four 512-token
           blocks (x tiles persist in SBUF).  v is stored per sk-tile as
           [v_h0 | 1] (65 cols) and [0*32 | 1 | 0*31 | v_h1] (128 cols)
           so each head's valuesT matmul lands its rows AND its softmax
           denominator at partition offsets that need no later shift.
  S(qb,pr):  per sk-tile/head: scoresT = kT.T @ qT -> exp -> valuesT
           accumulate.  exp runs on ACT (exact, fused 1/sqrt(hd) scale)
           or as a one-instruction Schraudolph exp2 bit trick on DVE
           (fp32*a+b -> int16 -> bitcast fp16), pattern-interleaved to
           split the exp wall across both engines.
  M: reciprocal of denominators (DVE) + PE ones-broadcast (f32r, reads
           the fp32 reciprocal directly) + DVE multiply into vals.
  O: o_part = valsT.T @ owT -> fp16 copy -> DMA out.
"""

import numpy as np

import concourse.bass as bass  # noqa: F401
import concourse.mybir as mybir
import concourse.tile as tile
from concourse import bacc
from concourse.bass_utils import run_bass_kernel_spmd

f32 = mybir.dt.float32
f32r = mybir.dt.float32r
fp16 = mybir.dt.float16
i16 = mybir.dt.int16
AF = mybir.ActivationFunctionType

import os as _os
MM_DT = fp16 if _os.environ.get("MM_DT", "fp16") == "fp16" else mybir.dt.bfloat16

P = 128
N, S, D, H = 2, 2048, 1024, 16
HD = D // H                    # 64
NH = 4                         # heads per core
SCALE = float(1.0 / np.sqrt(np.float32(HD)))
E_QK = 2 * NH * HD             # 512 qk rows per core
E_V = NH * HD                  # 256
DL = NH * HD                   # 256 local d for o_proj
SQB = 512                      # sq block
NSQB = S // SQB                # 4
SKT = S // P                   # 16 sk tiles
VSLOT = 200                    # v_sb per-(sk,pr) slot: h0 ext 65 + h1 ext 128

# Schraudolph exp2 constants (fp16 bits): u = round(z*EXA + EXB); u bitcast
# fp16 approximates exp(z*SCALE).  EXB centers the (1+f)/2^f sawtooth.
if _os.environ.get("MM_DT", "fp16") == "fp16":
    EXA = float(SCALE * np.log2(np.e) * 1024.0)
    EXB = float(15 << 10) + float(_os.environ.get("EXB_SHIFT", "-36.0"))
else:
    EXA = float(SCALE * np.log2(np.e) * 128.0)
    EXB = float(127 << 7) + float(_os.environ.get("EXB_SHIFT", "-4.5"))

# tuning knobs
RB = int(_os.environ.get("RB", "4"))          # shared PSUM ring bufs
ATTN_BUFS = int(_os.environ.get("ATTN_BUFS", "6"))
LAG = int(_os.environ.get("LAG", "2"))        # values matmul lag behind exp
# exp engine pattern per (sk-tile, head): A=ACT exact exp, D=DVE Schraudolph
EXP_PAT = _os.environ.get("EXP_PAT", "AD")
DEFER = int(_os.environ.get("DEFER", "1"))


def _emit_body(nc, tc, t, rep):
    from contextlib import ExitStack

    expi = [0]

    def emit_exp(at, sc):
        kind = EXP_PAT[expi[0] % len(EXP_PAT)]
        expi[0] += 1
        if kind == "A":
            nc.scalar.activation(at[:], sc[:], AF.Exp, scale=SCALE)
        else:
            nc.vector.tensor_scalar(
                out=at[:].bitcast(i16),
                in0=sc[:],
                scalar1=EXA,
                scalar2=EXB,
                op0=mybir.AluOpType.mult,
                op1=mybir.AluOpType.add,
            )

    cpi = [0]

    def cp(dst, src):
        if cpi[0] % 2 == 0:
            nc.scalar.copy(dst, src)
        else:
            nc.vector.tensor_copy(dst, src)
        cpi[0] += 1

    qs = [nc.gpsimd, nc.sync, nc.scalar]
    qi = [0]

    def q():
        e = qs[qi[0] % 3]
        qi[0] += 1
        return e

    with ExitStack() as ctx:
        const = ctx.enter_context(tc.tile_pool(name=f"const{rep}", bufs=1))
        persist = ctx.enter_context(tc.tile_pool(name=f"persist{rep}", bufs=1))

        wqk_sb = const.tile([P, 8, E_QK], MM_DT, name="wqk_sb")
        wv_sb = const.tile([P, 8, E_V], MM_DT, name="wv_sb")
        ow_sb = const.tile([P, 2, D], MM_DT, name="ow_sb")
        ones_sb = const.tile([65, HD], MM_DT, name="ones_sb")

        qT = persist.tile([P, 2, S], MM_DT, name="qT")
        kT = persist.tile([P, 2, S], MM_DT, name="kT")
        # v per sk-tile/pr: [0:65]   = [v_h0 | 1]
        #                  [72:200] = [0*32 | 1 | 0*31 | v_h1]
        v_sb = persist.tile([P, SKT, 2, VSLOT], MM_DT, name="v_sb")
        vals = persist.tile([P, 2, S], MM_DT, name="vals")
        xts = [persist.tile([P, 8, 512], MM_DT, name=f"xt{sb}")
               for sb in range(4)]

        # ---------------- startup DMAs (3 queues, earliest-needed first) --
        wqk_r = t["wqkT"].rearrange("(a p) e -> p a e", p=P)
        xT_r = t["xT"].rearrange("(a p) s -> p a s", p=P)
        q().dma_start(wv_sb[:], t["wvT"].rearrange("(a p) e -> p a e", p=P))
        # k-half of the weights first: pass A needs it immediately
        q().dma_start(wqk_sb[:, :, 128:256], wqk_r[:, :, 128:256])
        q().dma_start(wqk_sb[:, :, 384:512], wqk_r[:, :, 384:512])
        for sb in range(4):
            for ah in range(2):
                q().dma_start(xts[sb][:, 4 * ah:4 * ah + 4, :],
                              xT_r[:, 4 * ah:4 * ah + 4,
                                   sb * 512:(sb + 1) * 512])
        q().dma_start(wqk_sb[:, :, 0:128], wqk_r[:, :, 0:128])
        q().dma_start(wqk_sb[:, :, 256:384], wqk_r[:, :, 256:384])
        q().dma_start(ow_sb[:], t["owT"].rearrange("(a p) e -> p a e", p=P))
        # constants: ones rows for the denominator broadcasts + v_sb ones/zeros
        nc.vector.memset(ones_sb[32:33, :], 1.0)
        nc.vector.memset(ones_sb[64:65, :], 1.0)
        nc.vector.memset(v_sb[:, :, :, 64:65], 1.0)
        nc.vector.memset(v_sb[:, :, :, 104:105], 1.0)
        nc.vector.memset(v_sb[:, :, :, 72:136], 0.0)
        nc.vector.memset(v_sb[:, :, :, 104:105], 1.0)

        # ---------------- pass A: kT + v for all blocks ----------------
        with tc.tile_pool(name=f"psA{rep}", bufs=3, space="PSUM") as psA:
            for sb in range(4):
                ss = slice(sb * 512, (sb + 1) * 512)
                xt = xts[sb]
                for et in (1, 3):
                    pk = psA.tile([P, 512], f32, name="pk", tag="pk")
                    for a in range(8):
                        nc.tensor.matmul(
                            pk[:],
                            wqk_sb[:, a, et * 128:(et + 1) * 128],
                            xt[:, a, :],
                            start=(a == 0),
                            stop=(a == 7),
                        )
                    cp(kT[:, et // 2, ss], pk[:])
                for st in range(4):
                    pv = psA.tile([P, E_V], f32, name="pv", tag="pv", bufs=4)
                    for a in range(8):
                        nc.tensor.matmul(
                            pv[:],
                            xt[:, a, st * 128:(st + 1) * 128],
                            wv_sb[:, a, :],
                            start=(a == 0),
                            stop=(a == 7),
                        )
                    so = sb * 4 + st
                    pv_r = pv.rearrange("p (r h e) -> p r h e", r=2, h=2)
                    cp(v_sb[:, so, :, 0:64], pv_r[:, :, 0, :])
                    cp(v_sb[:, so, :, 136:200], pv_r[:, :, 1, :])

        # ---------------- fused S/M/O/qproj software pipeline ----------------
        with (
            tc.tile_pool(name=f"ps{rep}", bufs=RB, space="PSUM") as ps,
            tc.tile_pool(name=f"attn{rep}", bufs=ATTN_BUFS) as attnp,
            tc.tile_pool(name=f"sm{rep}", bufs=3) as sm,
            tc.tile_pool(name=f"outp{rep}", bufs=4) as outp,
        ):
            state = {}

            def qproj_et(sb, et):
                pq = ps.tile([P, 512], f32, name="pq", tag="ps")
                for a in range(8):
                    nc.tensor.matmul(
                        pq[:],
                        wqk_sb[:, a, et * 128:(et + 1) * 128],
                        xts[sb][:, a, :],
                        start=(a == 0),
                        stop=(a == 7),
                    )
                ss = slice(sb * 512, (sb + 1) * 512)
                cp(qT[:, et // 2, ss], pq[:])

            def m_bc(qb, pr, h):
                vps0, vps1, rec0, rec1 = state[(qb, pr)]
                bc = ps.tile([P, SQB], f32, name="bc", tag="ps")
                if h == 0:
                    nc.tensor.matmul(
                        bc[0:64, :],
                        ones_sb[64:65, :],
                        rec0[64:65, :],
                        start=True,
                        stop=True,
                    )
                else:
                    nc.tensor.matmul(
                        bc[64:128, :],
                        ones_sb[32:33, :],
                        rec1[32:33, :],
                        start=True,
                        stop=True,
                    )
                bcs = sm.tile([P, SQB], f32, name="bcs", tag="bcs", bufs=4)
                if h == 0:
                    cp(bcs[0:64, :], bc[0:64, :])
                else:
                    cp(bcs[64:128, :], bc[64:128, :])
                state[(qb, pr, "bc", h)] = bcs

            def m_mul(qb, pr, h):
                sqs = slice(qb * SQB, (qb + 1) * SQB)
                vps0, vps1, rec0, rec1 = state[(qb, pr)]
                bcs = state.pop((qb, pr, "bc", h))
                if h == 0:
                    nc.vector.tensor_mul(out=vals[0:64, pr, sqs],
                                         in0=vps0[0:64, :], in1=bcs[0:64, :])
                else:
                    nc.vector.tensor_mul(out=vals[64:128, pr, sqs],
                                         in0=vps1[64:128, :], in1=bcs[64:128, :])
                    state.pop((qb, pr))

            def o_tile(qb, st, eb):
                s0 = qb * 4 + st
                ops = ps.tile([P, 512], f32, name="ops", tag="ps")
                for a in range(2):
                    nc.tensor.matmul(
                        ops[:],
                        vals[:, a, s0 * 128:(s0 + 1) * 128],
                        ow_sb[:, a, eb * 512:(eb + 1) * 512],
                        start=(a == 0),
                        stop=(a == 1),
                    )
                ot = outp.tile([P, 512], MM_DT, name="ot")
                cp(ot[:], ops[:])
                nc.sync.dma_start(
                    t["o"][s0 * 128:(s0 + 1) * 128,
                           eb * 512:(eb + 1) * 512],
                    ot[:],
                )

            def Sblk(qb, pr, deferred):
                sqs = slice(qb * SQB, (qb + 1) * SQB)
                vps0 = ps.tile([65, SQB], f32, name="vps0", tag="vps0", bufs=2)
                vps1 = ps.tile([P, SQB], f32, name="vps1", tag="vps1", bufs=2)
                vv = (vps0, vps1)
                at_tiles = {}
                for g in range(SKT + LAG):
                    if g < SKT:
                        for h in range(2):
                            sc = ps.tile([P, SQB], f32, name="sc", tag="ps")
                            nc.tensor.matmul(
                                sc[:],
                                kT[h * 64:(h + 1) * 64, pr,
                                   g * 128:(g + 1) * 128],
                                qT[h * 64:(h + 1) * 64, pr, sqs],
                                start=True,
                                stop=True,
                            )
                            at = attnp.tile([P, SQB], MM_DT, name="at",
                                            tag="at")
                            emit_exp(at, sc)
                            at_tiles[(g, h)] = at
                    if g >= LAG:
                        gg = g - LAG
                        for h in range(2):
                            lo = 0 if h == 0 else 72
                            nc.tensor.matmul(
                                vv[h][:],
                                v_sb[:, gg, pr, lo:lo + (65 if h == 0 else 128)],
                                at_tiles.pop((gg, h))[:],
                                start=(gg == 0),
                                stop=(gg == SKT - 1),
                            )
                    for fn in deferred.get(g, ()):
                        fn()
                rec0 = sm.tile([65, SQB], f32, name="rec0", tag="rec0")
                nc.vector.reciprocal_approx_fast(rec0[0:65, :], vps0[0:65, :])
                rec1 = sm.tile([33, SQB], f32, name="rec1", tag="rec1")
                nc.vector.reciprocal_approx_fast(rec1[0:33, :], vps1[0:33, :])
                recr0 = sm.tile([65, SQB], MM_DT, name="recr0", tag="recr0")
                nc.scalar.copy(recr0[64:65, :], rec0[64:65, :])
                recr1 = sm.tile([33, SQB], MM_DT, name="recr1", tag="recr1")
                nc.scalar.copy(recr1[32:33, :], rec1[32:33, :])
                state[(qb, pr)] = (vps0, vps1, recr0, recr1)

            def mk(fn, *args):
                return lambda: fn(*args)

            # deferred-work schedules, one item per sk-tile position
            def defer_pr0(qb):
                d = {}
                if qb:
                    # normalization of (qb-1, pr1) + o_proj of qb-1
                    d[2] = (mk(m_bc, qb - 1, 1, 0),)
                    d[3] = (mk(m_bc, qb - 1, 1, 1),)
                    d[5] = (mk(m_mul, qb - 1, 1, 0),)
                    d[6] = (mk(m_mul, qb - 1, 1, 1),)
                    for i, g in enumerate((7, 8, 9, 10, 11, 13, 15, 16)):
                        d[g] = (mk(o_tile, qb - 1, i // 2, i % 2),)
                return d

            def defer_pr1(qb):
                d = {2: (mk(m_bc, qb, 0, 0),),
                     3: (mk(m_bc, qb, 0, 1),),
                     5: (mk(m_mul, qb, 0, 0),),
                     6: (mk(m_mul, qb, 0, 1),)}
                if qb < 3:
                    d[8] = (mk(qproj_et, qb + 1, 0),)
                    d[12] = (mk(qproj_et, qb + 1, 2),)
                return d

            if DEFER:
                qproj_et(0, 0)
                qproj_et(0, 2)
                for qb in range(NSQB):
                    Sblk(qb, 0, defer_pr0(qb))
                    Sblk(qb, 1, defer_pr1(qb))
                # tail: normalization of (3, pr1) + o_proj of qb 3
                m_bc(3, 1, 0)
                m_bc(3, 1, 1)
                m_mul(3, 1, 0)
                m_mul(3, 1, 1)
                for st in range(4):
                    for eb in range(2):
                        o_tile(3, st, eb)
            else:
                for qb in range(NSQB):
                    qproj_et(qb, 0)
                    qproj_et(qb, 2)
                    for pr in range(2):
                        Sblk(qb, pr, {})
                        m_bc(qb, pr, 0)
                        m_bc(qb, pr, 1)
                        m_mul(qb, pr, 0)
                        m_mul(qb, pr, 1)
                    for st in range(4):
                        for eb in range(2):
                            o_tile(qb, st, eb)


def build_nc(repeats: int = 1):
    nc = bacc.Bacc(None, target_bir_lowering=False)
    t = {
        "xT": nc.dram_tensor("xT", [D, S], MM_DT, kind="ExternalInput")[:, :],
        "wqkT": nc.dram_tensor("wqkT", [D, E_QK], MM_DT, kind="ExternalInput")[:, :],
        "wvT": nc.dram_tensor("wvT", [D, E_V], MM_DT, kind="ExternalInput")[:, :],
        "owT": nc.dram_tensor("owT", [DL, D], MM_DT, kind="ExternalInput")[:, :],
        "o": nc.dram_tensor("o", [S, D], MM_DT, kind="ExternalOutput")[:, :],
    }
    with tile.TileContext(nc) as tc:
        for rep in range(repeats):
            _emit_body(nc, tc, t, rep)
    nc.compile()
    return nc


def _f16(a):
    if MM_DT == fp16:
        return np.ascontiguousarray(a, dtype=np.float32).astype(np.float16)
    import ml_dtypes
    return np.ascontiguousarray(a, dtype=np.float32).astype(ml_dtypes.bfloat16)


def make_in_maps(x, qkv_w, o_w):
    x = np.ascontiguousarray(np.asarray(x, dtype=np.float32))
    qkv_w = np.ascontiguousarray(np.asarray(qkv_w, dtype=np.float32))
    o_w = np.ascontiguousarray(np.asarray(o_w, dtype=np.float32))
    in_maps = []
    for c in range(8):
        b, g = c // 4, c % 4
        heads = [4 * g + i for i in range(NH)]
        xT = np.ascontiguousarray(x[b].T)
        wq = [qkv_w[h * 192:h * 192 + 64] for h in heads]
        wk = [qkv_w[h * 192 + 64:h * 192 + 128] for h in heads]
        wv = [qkv_w[h * 192 + 128:h * 192 + 192] for h in heads]
        wqk = np.concatenate(
            [wq[0], wq[1], wk[0], wk[1], wq[2], wq[3], wk[2], wk[3]], axis=0
        )
        wqkT = np.ascontiguousarray(wqk.T)
        wvT = np.ascontiguousarray(np.concatenate(wv, axis=0).T)
        cols = np.concatenate([np.arange(h * 64, h * 64 + 64) for h in heads])
        owT = np.ascontiguousarray(o_w[:, cols].T)
        in_maps.append({"xT": _f16(xT), "wqkT": _f16(wqkT),
                        "wvT": _f16(wvT), "owT": _f16(owT)})
    return in_maps


_NC_CACHE = {}


def _get_nc(repeats=1):
    if repeats not in _NC_CACHE:
        _NC_CACHE[repeats] = build_nc(repeats)
    return _NC_CACHE[repeats]


def run_on_hw(x, qkv_w, o_w, repeats=1, **kwargs):
    nc = _get_nc(repeats)
    in_maps = make_in_maps(x, qkv_w, o_w)
    res = run_bass_kernel_spmd(nc, in_maps, core_ids=list(range(8)), **kwargs)
    out = np.zeros((N, S, D), dtype=np.float32)
    for c in range(8):
        out[c // 4] += np.asarray(res.results[c]["o"], dtype=np.float32)
    return out, res


def kernel(x, qkv_w, o_w):
    out, _ = run_on_hw(x, qkv_w, o_w)
    return out


# revision 25
# speedup vs baseline: 706.4916x; 1.0910x over previous
"""Multi-head attention (N=2, S=2048, D=1024, H=16) on 8 TRN2 NeuronCores.

Sharding: core c handles batch b = c//4 and head group g = c%4 (4 heads).
Column-parallel qkv projection, per-head attention, row-parallel o_proj;
the 4 partial o_proj outputs per batch are summed on the host (unshard).

Per-core device kernel (fp16 matmul operands, fp32 PSUM accumulate).
All engine queues are in-order, so emission order is the schedule: the
kernel is one long software pipeline where normalization (M), o_proj (O)
and the q-projection for the next block are woven one-item-per-sk-tile
into the scores/exp/values stream of a later S-block, giving every
cross-engine chain a full S-block of slack before the PE reaches its
dependent matmul.

  pass A:  kT = wkT.T @ xT and v = xT.T @ wvT for all four 512-token
           blocks (x tiles persist in SBUF).  v is stored per sk-tile as
           [v_h0 | 1] (65 cols) and [0*32 | 1 | 0*31 | v_h1] (128 cols)
           so each head's valuesT matmul lands its rows AND its softmax
           denominator at partition offsets that need no later shift.
  S(qb,pr):  per sk-tile/head: scoresT = kT.T @ qT -> exp -> valuesT
           accumulate.  exp runs on ACT (exact, fused 1/sqrt(hd) scale)
           or as a one-instruction Schraudolph exp2 bit trick on DVE
           (fp32*a+b -> int16 -> bitcast fp16), pattern-interleaved to
           split the exp wall across both engines.
  M: reciprocal of denominators (DVE) + PE ones-broadcast (f32r, reads
           the fp32 reciprocal directly) + DVE multiply into vals.
  O: o_part = valsT.T @ owT -> fp16 copy -> DMA out.
"""

import numpy as np

import concourse.bass as bass  # noqa: F401
import concourse.mybir as mybir
import concourse.tile as tile
from concourse import bacc
from concourse.bass_utils import run_bass_kernel_spmd

f32 = mybir.dt.float32
f32r = mybir.dt.float32r
fp16 = mybir.dt.float16
i16 = mybir.dt.int16
AF = mybir.ActivationFunctionType

import os as _os
MM_DT = fp16 if _os.environ.get("MM_DT", "fp16") == "fp16" else mybir.dt.bfloat16

P = 128
N, S, D, H = 2, 2048, 1024, 16
HD = D // H                    # 64
NH = 4                         # heads per core
SCALE = float(1.0 / np.sqrt(np.float32(HD)))
E_QK = 2 * NH * HD             # 512 qk rows per core
E_V = NH * HD                  # 256
DL = NH * HD                   # 256 local d for o_proj
SQB = 512                      # sq block
NSQB = S // SQB                # 4
SKT = S // P                   # 16 sk tiles
VSLOT = 200                    # v_sb per-(sk,pr) slot: h0 ext 65 + h1 ext 128

# Schraudolph exp2 constants (fp16 bits): u = round(z*EXA + EXB); u bitcast
# fp16 approximates exp(z*SCALE).  EXB centers the (1+f)/2^f sawtooth.
if _os.environ.get("MM_DT", "fp16") == "fp16":
    EXA = float(SCALE * np.log2(np.e) * 1024.0)
    EXB = float(15 << 10) + float(_os.environ.get("EXB_SHIFT", "-36.0"))
else:
    EXA = float(SCALE * np.log2(np.e) * 128.0)
    EXB = float(127 << 7) + float(_os.environ.get("EXB_SHIFT", "-4.5"))

# tuning knobs
RB = int(_os.environ.get("RB", "4"))          # shared PSUM ring bufs
ATTN_BUFS = int(_os.environ.get("ATTN_BUFS", "6"))
LAG = int(_os.environ.get("LAG", "2"))        # values matmul lag behind exp
# exp engine pattern per (sk-tile, head): A=ACT exact exp, D=DVE Schraudolph
EXP_PAT = _os.environ.get("EXP_PAT", "ADAAD")
DEFER = int(_os.environ.get("DEFER", "1"))


def _emit_body(nc, tc, t, rep):
    from contextlib import ExitStack

    expi = [0]

    def emit_exp(at, sc):
        kind = EXP_PAT[expi[0] % len(EXP_PAT)]
        expi[0] += 1
        if kind == "A":
            nc.scalar.activation(at[:], sc[:], AF.Exp, scale=SCALE)
        else:
            nc.vector.tensor_scalar(
                out=at[:].bitcast(i16),
                in0=sc[:],
                scalar1=EXA,
                scalar2=EXB,
                op0=mybir.AluOpType.mult,
                op1=mybir.AluOpType.add,
            )

    cpi = [0]

    def cp(dst, src):
        if cpi[0] % 2 == 0:
            nc.scalar.copy(dst, src)
        else:
            nc.vector.tensor_copy(dst, src)
        cpi[0] += 1

    qs = [nc.gpsimd, nc.sync, nc.scalar]
    qi = [0]

    def q():
        e = qs[qi[0] % 3]
        qi[0] += 1
        return e

    with ExitStack() as ctx:
        const = ctx.enter_context(tc.tile_pool(name=f"const{rep}", bufs=1))
        persist = ctx.enter_context(tc.tile_pool(name=f"persist{rep}", bufs=1))

        wqk_sb = const.tile([P, 8, E_QK], MM_DT, name="wqk_sb")
        wv_sb = const.tile([P, 8, E_V], MM_DT, name="wv_sb")
        ow_sb = const.tile([P, 2, D], MM_DT, name="ow_sb")
        ones_sb = const.tile([65, HD], MM_DT, name="ones_sb")

        qT = persist.tile([P, 2, S], MM_DT, name="qT")
        kT = persist.tile([P, 2, S], MM_DT, name="kT")
        # v per sk-tile/pr: [0:65]   = [v_h0 | 1]
        #                  [72:200] = [0*32 | 1 | 0*31 | v_h1]
        v_sb = persist.tile([P, SKT, 2, VSLOT], MM_DT, name="v_sb")
        vals = persist.tile([P, 2, S], MM_DT, name="vals")
        xts = [persist.tile([P, 8, 512], MM_DT, name=f"xt{sb}")
               for sb in range(4)]

        # ---------------- startup DMAs (3 queues, earliest-needed first) --
        wqk_r = t["wqkT"].rearrange("(a p) e -> p a e", p=P)
        xT_r = t["xT"].rearrange("(a p) s -> p a s", p=P)
        # k-half of the weights + first x block first: pass A needs them now
        nc.gpsimd.dma_start(wqk_sb[:, :, 128:256], wqk_r[:, :, 128:256])
        nc.sync.dma_start(wqk_sb[:, :, 384:512], wqk_r[:, :, 384:512])
        for ah in range(4):
            q().dma_start(xts[0][:, 2 * ah:2 * ah + 2, :],
                          xT_r[:, 2 * ah:2 * ah + 2, 0:512])
        nc.scalar.dma_start(wv_sb[:], t["wvT"].rearrange("(a p) e -> p a e", p=P))
        for sb in range(1, 4):
            for ah in range(2):
                q().dma_start(xts[sb][:, 4 * ah:4 * ah + 4, :],
                              xT_r[:, 4 * ah:4 * ah + 4,
                                   sb * 512:(sb + 1) * 512])
        q().dma_start(wqk_sb[:, :, 0:128], wqk_r[:, :, 0:128])
        q().dma_start(wqk_sb[:, :, 256:384], wqk_r[:, :, 256:384])
        q().dma_start(ow_sb[:], t["owT"].rearrange("(a p) e -> p a e", p=P))
        # constants: ones rows for the denominator broadcasts + v_sb ones/zeros
        nc.vector.memset(ones_sb[32:33, :], 1.0)
        nc.vector.memset(ones_sb[64:65, :], 1.0)
        nc.vector.memset(v_sb[:, :, :, 64:65], 1.0)
        nc.vector.memset(v_sb[:, :, :, 104:105], 1.0)
        nc.vector.memset(v_sb[:, :, :, 72:136], 0.0)
        nc.vector.memset(v_sb[:, :, :, 104:105], 1.0)

        # ---------------- fused pipeline: one PSUM pool for everything ----
        with (
            tc.tile_pool(name=f"ps{rep}", bufs=RB, space="PSUM") as ps,
            tc.tile_pool(name=f"attn{rep}", bufs=ATTN_BUFS) as attnp,
            tc.tile_pool(name=f"sm{rep}", bufs=3) as sm,
            tc.tile_pool(name=f"outp{rep}", bufs=4) as outp,
        ):
            state = {}

            # pass A: kT + v for all blocks
            for sb in range(4):
                ss = slice(sb * 512, (sb + 1) * 512)
                xt = xts[sb]
                for et in (1, 3):
                    pk = ps.tile([P, 512], f32, name="pk", tag="ps")
                    for a in range(8):
                        nc.tensor.matmul(
                            pk[:],
                            wqk_sb[:, a, et * 128:(et + 1) * 128],
                            xt[:, a, :],
                            start=(a == 0),
                            stop=(a == 7),
                        )
                    cp(kT[:, et // 2, ss], pk[:])
                for st in range(4):
                    pv = ps.tile([P, E_V], f32, name="pv", tag="ps")
                    for a in range(8):
                        nc.tensor.matmul(
                            pv[:],
                            xt[:, a, st * 128:(st + 1) * 128],
                            wv_sb[:, a, :],
                            start=(a == 0),
                            stop=(a == 7),
                        )
                    so = sb * 4 + st
                    pv_r = pv.rearrange("p (r h e) -> p r h e", r=2, h=2)
                    cp(v_sb[:, so, :, 0:64], pv_r[:, :, 0, :])
                    cp(v_sb[:, so, :, 136:200], pv_r[:, :, 1, :])

            def qproj_et(sb, et):
                pq = ps.tile([P, 512], f32, name="pq", tag="ps")
                for a in range(8):
                    nc.tensor.matmul(
                        pq[:],
                        wqk_sb[:, a, et * 128:(et + 1) * 128],
                        xts[sb][:, a, :],
                        start=(a == 0),
                        stop=(a == 7),
                    )
                ss = slice(sb * 512, (sb + 1) * 512)
                cp(qT[:, et // 2, ss], pq[:])

            def m_recip(qb, pr, h):
                st_ = state[(qb, pr)]
                if h == 0:
                    rec0 = sm.tile([65, SQB], f32, name="rec0", tag="rec0")
                    nc.vector.reciprocal_approx_fast(rec0[0:65, :],
                                                     st_[0][0:65, :])
                    st_[2] = rec0
                else:
                    rec1 = sm.tile([33, SQB], f32, name="rec1", tag="rec1")
                    nc.vector.reciprocal_approx_fast(rec1[0:33, :],
                                                     st_[1][0:33, :])
                    st_[3] = rec1

            def m_recr(qb, pr):
                st_ = state[(qb, pr)]
                recr0 = sm.tile([65, SQB], MM_DT, name="recr0", tag="recr0")
                nc.gpsimd.tensor_copy(recr0[64:65, :], st_[2][64:65, :])
                recr1 = sm.tile([33, SQB], MM_DT, name="recr1", tag="recr1")
                nc.gpsimd.tensor_copy(recr1[32:33, :], st_[3][32:33, :])
                st_[2] = recr0
                st_[3] = recr1

            def m_bc(qb, pr, h):
                vps0, vps1, rec0, rec1 = state[(qb, pr)]
                bc = ps.tile([P, SQB], f32, name="bc", tag="ps")
                if h == 0:
                    nc.tensor.matmul(
                        bc[0:64, :],
                        ones_sb[64:65, :],
                        rec0[64:65, :],
                        start=True,
                        stop=True,
                    )
                else:
                    nc.tensor.matmul(
                        bc[64:128, :],
                        ones_sb[32:33, :],
                        rec1[32:33, :],
                        start=True,
                        stop=True,
                    )
                bcs = sm.tile([P, SQB], f32, name="bcs", tag="bcs", bufs=4)
                if h == 0:
                    cp(bcs[0:64, :], bc[0:64, :])
                else:
                    cp(bcs[64:128, :], bc[64:128, :])
                state[(qb, pr, "bc", h)] = bcs

            def m_mul(qb, pr, h):
                sqs = slice(qb * SQB, (qb + 1) * SQB)
                vps0, vps1, rec0, rec1 = state[(qb, pr)]
                bcs = state.pop((qb, pr, "bc", h))
                if h == 0:
                    nc.vector.tensor_mul(out=vals[0:64, pr, sqs],
                                         in0=vps0[0:64, :], in1=bcs[0:64, :])
                else:
                    nc.vector.tensor_mul(out=vals[64:128, pr, sqs],
                                         in0=vps1[64:128, :], in1=bcs[64:128, :])
                    state.pop((qb, pr))

            def o_tile(qb, st, eb):
                s0 = qb * 4 + st
                ops = ps.tile([P, 512], f32, name="ops", tag="ps")
                for a in range(2):
                    nc.tensor.matmul(
                        ops[:],
                        vals[:, a, s0 * 128:(s0 + 1) * 128],
                        ow_sb[:, a, eb * 512:(eb + 1) * 512],
                        start=(a == 0),
                        stop=(a == 1),
                    )
                ot = outp.tile([P, 512], MM_DT, name="ot")
                cp(ot[:], ops[:])
                nc.sync.dma_start(
                    t["o"][s0 * 128:(s0 + 1) * 128,
                           eb * 512:(eb + 1) * 512],
                    ot[:],
                )

            def Sblk(qb, pr, deferred):
                sqs = slice(qb * SQB, (qb + 1) * SQB)
                vps0 = ps.tile([65, SQB], f32, name="vps0", tag="vps0", bufs=2)
                vps1 = ps.tile([P, SQB], f32, name="vps1", tag="vps1", bufs=2)
                vv = (vps0, vps1)
                at_tiles = {}
                for g in range(SKT + LAG):
                    if g < SKT:
                        for h in range(2):
                            sc = ps.tile([P, SQB], f32, name="sc", tag="ps")
                            nc.tensor.matmul(
                                sc[:],
                                kT[h * 64:(h + 1) * 64, pr,
                                   g * 128:(g + 1) * 128],
                                qT[h * 64:(h + 1) * 64, pr, sqs],
                                start=True,
                                stop=True,
                            )
                            at = attnp.tile([P, SQB], MM_DT, name="at",
                                            tag="at")
                            emit_exp(at, sc)
                            at_tiles[(g, h)] = at
                    if g >= LAG:
                        gg = g - LAG
                        for h in range(2):
                            lo = 0 if h == 0 else 72
                            nc.tensor.matmul(
                                vv[h][:],
                                v_sb[:, gg, pr, lo:lo + (65 if h == 0 else 128)],
                                at_tiles.pop((gg, h))[:],
                                start=(gg == 0),
                                stop=(gg == SKT - 1),
                            )
                    for fn in deferred.get(g, ()):
                        fn()
                state[(qb, pr)] = [vps0, vps1, None, None]

            def mk(fn, *args):
                return lambda: fn(*args)

            # deferred-work schedules, one item per sk-tile position
            def defer_pr0(qb):
                d = {}
                if qb:
                    # normalization of (qb-1, pr1) + o_proj of qb-1
                    d[2] = (mk(m_recip, qb - 1, 1, 0),)
                    d[3] = (mk(m_recip, qb - 1, 1, 1),)
                    d[4] = (mk(m_recr, qb - 1, 1),)
                    d[5] = (mk(m_bc, qb - 1, 1, 0),)
                    d[6] = (mk(m_bc, qb - 1, 1, 1),)
                    d[7] = (mk(m_mul, qb - 1, 1, 0),)
                    d[8] = (mk(m_mul, qb - 1, 1, 1),)
                    for i, g in enumerate((9, 10, 11, 12, 13, 14, 15, 16)):
                        d[g] = d.get(g, ()) + (mk(o_tile, qb - 1, i // 2, i % 2),)
                return d

            def defer_pr1(qb):
                d = {2: (mk(m_recip, qb, 0, 0),),
                     3: (mk(m_recip, qb, 0, 1),),
                     4: (mk(m_recr, qb, 0),),
                     5: (mk(m_bc, qb, 0, 0),),
                     6: (mk(m_bc, qb, 0, 1),),
                     7: (mk(m_mul, qb, 0, 0),),
                     8: (mk(m_mul, qb, 0, 1),)}
                if qb < 3:
                    d[10] = (mk(qproj_et, qb + 1, 0),)
                    d[14] = (mk(qproj_et, qb + 1, 2),)
                return d

            if DEFER:
                qproj_et(0, 0)
                qproj_et(0, 2)
                for qb in range(NSQB):
                    Sblk(qb, 0, defer_pr0(qb))
                    Sblk(qb, 1, defer_pr1(qb))
                # tail: normalization of (3, pr1) + o_proj of qb 3
                m_recip(3, 1, 0)
                m_recip(3, 1, 1)
                m_recr(3, 1)
                m_bc(3, 1, 0)
                m_bc(3, 1, 1)
                m_mul(3, 1, 0)
                m_mul(3, 1, 1)
                for st in range(4):
                    for eb in range(2):
                        o_tile(3, st, eb)
            else:
                for qb in range(NSQB):
                    qproj_et(qb, 0)
                    qproj_et(qb, 2)
                    for pr in range(2):
                        Sblk(qb, pr, {})
                        m_recip(qb, pr, 0)
                        m_recip(qb, pr, 1)
                        m_recr(qb, pr)
                        m_bc(qb, pr, 0)
                        m_bc(qb, pr, 1)
                        m_mul(qb, pr, 0)
                        m_mul(qb, pr, 1)
                    for st in range(4):
                        for eb in range(2):
                            o_tile(qb, st, eb)


def build_nc(repeats: int = 1):
    nc = bacc.Bacc(None, target_bir_lowering=False)
    t = {
        "xT": nc.dram_tensor("xT", [D, S], MM_DT, kind="ExternalInput")[:, :],
        "wqkT": nc.dram_tensor("wqkT", [D, E_QK], MM_DT, kind="ExternalInput")[:, :],
        "wvT": nc.dram_tensor("wvT", [D, E_V], MM_DT, kind="ExternalInput")[:, :],
        "owT": nc.dram_tensor("owT", [DL, D], MM_DT, kind="ExternalInput")[:, :],
        "o": nc.dram_tensor("o", [S, D], MM_DT, kind="ExternalOutput")[:, :],
    }
    with tile.TileContext(nc) as tc:
        for rep in range(repeats):
            _emit_body(nc, tc, t, rep)
    nc.compile()
    return nc


def _f16(a):
    if MM_DT == fp16:
        return np.ascontiguousarray(a, dtype=np.float32).astype(np.float16)
    import ml_dtypes
    return np.ascontiguousarray(a, dtype=np.float32).astype(ml_dtypes.bfloat16)


def make_in_maps(x, qkv_w, o_w):
    x = np.ascontiguousarray(np.asarray(x, dtype=np.float32))
    qkv_w = np.ascontiguousarray(np.asarray(qkv_w, dtype=np.float32))
    o_w = np.ascontiguousarray(np.asarray(o_w, dtype=np.float32))
    in_maps = []
    for c in range(8):
        b, g = c // 4, c % 4
        heads = [4 * g + i for i in range(NH)]
        xT = np.ascontiguousarray(x[b].T)
        wq = [qkv_w[h * 192:h * 192 + 64] for h in heads]
        wk = [qkv_w[h * 192 + 64:h * 192 + 128] for h in heads]
        wv = [qkv_w[h * 192 + 128:h * 192 + 192] for h in heads]
        wqk = np.concatenate(
            [wq[0], wq[1], wk[0], wk[1], wq[2], wq[3], wk[2], wk[3]], axis=0
        )
        wqkT = np.ascontiguousarray(wqk.T)
        wvT = np.ascontiguousarray(np.concatenate(wv, axis=0).T)
        cols = np.concatenate([np.arange(h * 64, h * 64 + 64) for h in heads])
        owT = np.ascontiguousarray(o_w[:, cols].T)
        in_maps.append({"xT": _f16(xT), "wqkT": _f16(wqkT),
                        "wvT": _f16(wvT), "owT": _f16(owT)})
    return in_maps


_NC_CACHE = {}


def _get_nc(repeats=1):
    if repeats not in _NC_CACHE:
        _NC_CACHE[repeats] = build_nc(repeats)
    return _NC_CACHE[repeats]


def run_on_hw(x, qkv_w, o_w, repeats=1, **kwargs):
    nc = _get_nc(repeats)
    in_maps = make_in_maps(x, qkv_w, o_w)
    res = run_bass_kernel_spmd(nc, in_maps, core_ids=list(range(8)), **kwargs)
    out = np.zeros((N, S, D), dtype=np.float32)
    for c in range(8):
        out[c // 4] += np.asarray(res.results[c]["o"], dtype=np.float32)
    return out, res


def kernel(x, qkv_w, o_w):
    out, _ = run_on_hw(x, qkv_w, o_w)
    return out
